# revision 1
# baseline (speedup 1.0000x reference)
"""SpMM (COO segment-sum) kernel for trn2, 8 NeuronCores.

out[i] = sum_{e: row[e]==i} val[e] * x[col[e]]   (N=65536, E~1M, D=64)

Strategy (dest-row 1D sharding, per spec hint):
- Host: stable-sort edges by destination row; shard rows 8192/core; bucket
  edges into 64-row windows, split into two column streams (col<32768 /
  col>=32768 so node indices fit dma_gather's int16); pad each bucket to
  whole 128-edge blocks (pad: idx=0, val=0). Block counts are maxed across
  cores so all 8 cores run one SPMD program. Windows are processed in
  batches of 8; each batch's blocks are gathered in up-to-1024-index
  dma_gather chunks rotating over 4 SWDGE queues.
- Device, per chunk (<=8 blocks):
    g[p, k, :]  = x[colidx[p + 128 k]]               (dma_gather)
    eq[p, kW+r] = (iota_r == row_local[p, k])        (DVE, batched)
    sel         = eq * val[p, k]                     (DVE, batched)
  per block j:  psum_w[r, f] += sum_p sel[p, jW+r] * g[p, j, f]  (PE)
  PSUM windows drain via ACT copy to SBUF, batched DMA to out.
"""

import os
import numpy as np

N_NODES = 65536
D = 64
P = 128
N_CORES = 8
ROWS_PER_CORE = N_NODES // N_CORES   # 8192
W = 64                               # rows per PSUM window
WINDOWS = ROWS_PER_CORE // W         # 128
G_W = 8                              # windows per batch (PSUM live set)
HALF = N_NODES // 2                  # int16-addressable half
CHUNK_BLOCKS = 8                     # <=1024 idxs per dma_gather
NQ = 4                               # SWDGE queues
OUT_BATCH = 16                       # windows per output DMA

LAST_EXEC_NS = None


def _o_index(w, s):
    return (w // G_W) * (2 * G_W) + s * G_W + (w % G_W)


def _pack(row, col, val):
    """Host-side packing. Returns per-core device arrays + shared program map."""
    E = row.shape[0]
    core = row // ROWS_PER_CORE
    win = (row % ROWS_PER_CORE) // W
    strm = (col >= HALF).astype(np.int64)
    NG = WINDOWS * 2

    gkey = _o_index(win, strm)
    order = np.lexsort((row, gkey, core))
    rs, cs, vs, gs, cos = (row[order], col[order], val[order],
                           gkey[order], core[order])

    cnt = np.zeros((N_CORES, NG), np.int64)
    np.add.at(cnt, (cos, gs), 1)
    B = -(-cnt // P).max(axis=0) * -1       # ceil then max: see below
    B = (-(-cnt // P)).max(axis=0)          # [NG] blocks per group, shared
    # every window needs >=1 block so its PSUM window is written
    for w in range(WINDOWS):
        oL, oH = _o_index(w, 0), _o_index(w, 1)
        if B[oL] + B[oH] == 0:
            B[oL] = 1
    group_base = np.zeros(NG + 1, np.int64)
    np.cumsum(B * P, out=group_base[1:])
    total_blocks = int(B.sum())
    slots = total_blocks * P

    # per-edge slot position
    ckey = cos * NG + gs
    starts = np.zeros(E, np.int64)
    newgrp = np.ones(E, bool)
    newgrp[1:] = ckey[1:] != ckey[:-1]
    start_idx = np.where(newgrp)[0]
    starts[start_idx] = start_idx
    starts = np.maximum.accumulate(starts)
    rank = np.arange(E) - starts
    pos = group_base[gs] + rank

    idxf = np.zeros((N_CORES, slots), np.int16)
    rowf = np.zeros((N_CORES, slots), np.float32)
    valf = np.zeros((N_CORES, slots), np.float32)
    cidx = np.where(gs % 16 < G_W, cs, cs - HALF).astype(np.int16)
    idxf[cos, pos] = cidx
    rowf[cos, pos] = (rs % W).astype(np.float32)
    valf[cos, pos] = vs

    # block -> (window, stream) map in slot order
    blk_w = np.zeros(total_blocks, np.int64)
    blk_s = np.zeros(total_blocks, np.int64)
    bp = 0
    group_of_o = []
    for o in range(NG):
        b_ = o % (2 * G_W)
        s_ = b_ // G_W
        w_ = (o // (2 * G_W)) * G_W + (b_ % G_W)
        group_of_o.append((w_, s_))
        blk_w[bp:bp + B[o]] = w_
        blk_s[bp:bp + B[o]] = s_
        bp += B[o]

    # chunks: consecutive blocks of one (batch, stream) section, <=CHUNK_BLOCKS
    chunks = []   # (bat, s, blk_base, nblk, col_base)
    col_ptr = 0
    bp = 0
    for bat in range(WINDOWS // G_W):
        for s_ in range(2):
            o0 = bat * 2 * G_W + s_ * G_W
            sec_blocks = int(B[o0:o0 + G_W].sum())
            done = 0
            while done < sec_blocks:
                nb = min(CHUNK_BLOCKS, sec_blocks - done)
                chunks.append((bat, s_, bp, nb, col_ptr))
                col_ptr += nb * 8
                bp += nb
                done += nb
    S_tot = col_ptr

    # first/last block per window (for matmul start/stop)
    win_first = np.full(WINDOWS, -1, np.int64)
    win_last = np.zeros(WINDOWS, np.int64)
    for b in range(total_blocks):
        w_ = blk_w[b]
        if win_first[w_] < 0:
            win_first[w_] = b
        win_last[w_] = b

    # idx wrapped layout per chunk, x8 replicated
    idx2d = np.zeros((N_CORES, 16, S_tot), np.int16)
    for (_, s_, bb, nb, cb) in chunks:
        seg = idxf[:, bb * P:(bb + nb) * P]
        idx2d[:, :, cb:cb + nb * 8] = seg.reshape(N_CORES, nb * 8, 16).transpose(0, 2, 1)
    idx2d = np.tile(idx2d, (1, 8, 1))

    rowt = rowf.reshape(N_CORES, total_blocks, P).transpose(0, 2, 1).copy()
    valt = valf.reshape(N_CORES, total_blocks, P).transpose(0, 2, 1).copy()
    return (idx2d, rowt, valt, chunks, blk_w, win_first, win_last,
            total_blocks, S_tot)


def _build(chunks, blk_w, win_first, win_last, total_blocks, S_tot):
    import concourse.bacc as bacc
    import concourse.mybir as mybir
    from concourse.tile import TileContext

    nc = bacc.Bacc("TRN2", target_bir_lowering=False, debug=False,
                   num_swdge_queues=NQ)
    f32 = mybir.dt.float32
    xlo = nc.dram_tensor("xlo", [HALF, D], f32, kind="ExternalInput")
    xhi = nc.dram_tensor("xhi", [HALF, D], f32, kind="ExternalInput")
    idxs = nc.dram_tensor("idxs", [P, S_tot], mybir.dt.int16, kind="ExternalInput")
    rowd = nc.dram_tensor("rowt", [P, total_blocks], f32, kind="ExternalInput")
    vald = nc.dram_tensor("valt", [P, total_blocks], f32, kind="ExternalInput")
    out = nc.dram_tensor("out", [ROWS_PER_CORE, D], f32, kind="ExternalOutput")
    xsrc = (xlo, xhi)

    with TileContext(nc) as tc:
        with (
            tc.tile_pool(name="meta", bufs=1) as meta,
            tc.tile_pool(name="gat", bufs=8) as gat,
            tc.tile_pool(name="selp", bufs=8) as selp,
            tc.tile_pool(name="psum", bufs=8, space="PSUM") as psp,
            tc.tile_pool(name="ost", bufs=2) as ostp,
        ):
            idx_tile = meta.tile([P, S_tot], mybir.dt.int16)
            # split the idx load so early gathers start before the whole
            # table has landed (Tile tracks sub-range deps)
            n_split = 4
            step = -(-S_tot // n_split)
            for si in range(n_split):
                a, b_ = si * step, min((si + 1) * step, S_tot)
                if a < b_:
                    nc.sync.dma_start(out=idx_tile[:, a:b_], in_=idxs[:, a:b_])
            row_tile = meta.tile([P, total_blocks], f32)
            nc.sync.dma_start(out=row_tile[:], in_=rowd[:, :])
            val_tile = meta.tile([P, total_blocks], f32)
            nc.sync.dma_start(out=val_tile[:], in_=vald[:, :])
            iota_i = meta.tile([P, CHUNK_BLOCKS * W], mybir.dt.int32)
            nc.gpsimd.iota(iota_i[:], pattern=[[0, CHUNK_BLOCKS], [1, W]],
                           base=0, channel_multiplier=0)
            iota_f = meta.tile([P, CHUNK_BLOCKS * W], f32)
            nc.vector.tensor_copy(out=iota_f[:], in_=iota_i[:])

            def drain_batch(bat):
                nonlocal out_stage
                for w_ in range(bat * G_W, (bat + 1) * G_W):
                    wi = w_ % OUT_BATCH
                    if wi == 0:
                        out_stage = ostp.tile([W, OUT_BATCH * D], f32)
                    nc.scalar.copy(out=out_stage[:, wi * D:(wi + 1) * D],
                                   in_=psum_of.pop(w_)[:, :])
                    if wi == OUT_BATCH - 1:
                        w0 = w_ - (OUT_BATCH - 1)
                        dview = out[w0 * W:(w_ + 1) * W, :].rearrange(
                            "(g p) f -> p g f", p=W)
                        sview = out_stage[:].rearrange("p (g f) -> p g f", f=D)
                        nc.sync.dma_start(out=dview, in_=sview)

            psum_of = {}
            out_stage = None
            cur_bat = 0
            qi = 0
            for (bat, s_, bb, nb, cb) in chunks:
                if bat != cur_bat:
                    drain_batch(cur_bat)
                    cur_bat = bat
                g = gat.tile([P, CHUNK_BLOCKS * D], f32, tag="g")
                nc.gpsimd.dma_gather(
                    out_ap=g[:, :nb * D].rearrange("p (k d) -> p k d", d=D),
                    in_ap=xsrc[s_][:],
                    idxs_ap=idx_tile[:, cb:cb + nb * 8],
                    num_idxs=nb * P,
                    num_idxs_reg=nb * P,
                    elem_size=D,
                    queue_num=qi % NQ,
                )
                qi += 1

                selt = selp.tile([P, CHUNK_BLOCKS * W], f32, tag="sel")
                sel3 = selt[:, :nb * W].rearrange("p (k w) -> p k w", w=W)
                nc.vector.tensor_tensor(
                    out=sel3,
                    in0=iota_f[:, :nb * W].rearrange("p (k w) -> p k w", w=W),
                    in1=row_tile[:, bb:bb + nb].to_broadcast([P, nb, W]),
                    op=mybir.AluOpType.is_equal,
                )
                nc.vector.tensor_tensor(
                    out=sel3,
                    in0=sel3,
                    in1=val_tile[:, bb:bb + nb].to_broadcast([P, nb, W]),
                    op=mybir.AluOpType.mult,
                )

                for j in range(nb):
                    b = bb + j
                    w_ = int(blk_w[b])
                    if w_ not in psum_of:
                        psum_of[w_] = psp.tile([W, D], f32, name='psw', tag='psw')
                    nc.tensor.matmul(
                        out=psum_of[w_][:, :],
                        lhsT=selt[:, j * W:(j + 1) * W],
                        rhs=g[:, j * D:(j + 1) * D],
                        start=(b == win_first[w_]),
                        stop=(b == win_last[w_]),
                    )
            drain_batch(cur_bat)
    nc.compile()
    return nc


def kernel(x, row, col, val, idx):
    global LAST_EXEC_NS
    from concourse.bass_utils import run_bass_kernel_spmd

    x = np.ascontiguousarray(np.asarray(x), dtype=np.float32)
    row = np.asarray(row).astype(np.int64)
    col = np.asarray(col).astype(np.int64)
    val = np.ascontiguousarray(np.asarray(val), dtype=np.float32)

    (idx2d, rowt, valt, chunks, blk_w, win_first, win_last,
     total_blocks, S_tot) = _pack(row, col, val)
    nc = _build(chunks, blk_w, win_first, win_last, total_blocks, S_tot)

    xlo = np.ascontiguousarray(x[:HALF])
    xhi = np.ascontiguousarray(x[HALF:])
    in_maps = [
        {"xlo": xlo, "xhi": xhi, "idxs": idx2d[c], "rowt": rowt[c],
         "valt": valt[c]}
        for c in range(N_CORES)
    ]
    trace = os.environ.get("BASS_KERNEL_TRACE", "0") == "1"
    res = run_bass_kernel_spmd(nc, in_maps, list(range(N_CORES)), trace=trace)
    LAST_EXEC_NS = res.exec_time_ns
    outs = [np.asarray(res.results[c]["out"]) for c in range(N_CORES)]
    return np.concatenate(outs, axis=0)



# revision 7
# speedup vs baseline: 1.2156x; 1.2156x over previous
"""SpMM (COO segment-sum) kernel for trn2, 8 NeuronCores.

out[i] = sum_{e: row[e]==i} val[e] * x[col[e]]   (N=65536, E~1M, D=64)

Strategy (dest-row 1D sharding per spec hint), v2:
- Host: shard rows 8192/core; within a core, bucket edges into 16
  batches of 512 rows x 2 column streams (col<32768 / col>=32768 so
  node indices fit dma_gather's int16), sort each (batch, stream)
  group by row and pack densely into 128-edge blocks (pad only the
  group tail; block counts maxed across cores so all 8 cores run one
  SPMD program -> ~6% padding). x is stored bf16 padded to 128 feats
  so each gathered row is 256B (dma_gather's granularity floor).
- A block's rows may span several 64-row PSUM windows. Host emits
  (block, window) "tasks" as the union of spans across cores; on a
  core where the block misses the window, row-rel values fall outside
  [0,64) and the one-hot select is all zero, so the matmul is a no-op.
- Device, per (batch, stream) group: ONE dma_gather for the whole
  group (amortizes the ~1us SWDGE fixed overhead), then per task:
    sel[p, t*64+r] = (iota_r == rowT[p, t]) * valT[p, t]   (DVE, bf16,
                      batched over the group's tasks)
    psum[w][r, f] += sum_p sel[p, t*64+r] * g[p, f]        (PE, bf16)
  PSUM windows drain via ACT copy to SBUF, one output DMA per batch.
"""

import os
import numpy as np
import ml_dtypes

N_NODES = 65536
D = 64
DP = 128                             # padded feature dim (256B bf16 rows)
P = 128
N_CORES = 8
ROWS_PER_CORE = N_NODES // N_CORES   # 8192
W = 64                               # rows per PSUM window
WINDOWS = ROWS_PER_CORE // W         # 128
WPB = 8                              # windows per batch
BR = W * WPB                         # rows per batch (512)
NBATCH = ROWS_PER_CORE // BR         # 16
NG = NBATCH * 2                      # (batch, stream) groups
HALF = N_NODES // 2
NQ = 4                               # SWDGE queues
CH_BLOCKS = 8                        # blocks per dma_gather chunk

LAST_EXEC_NS = None


def _pack(row, col, val):
    """Host-side packing. Returns per-core device arrays + shared program."""
    E = row.shape[0]
    core = row // ROWS_PER_CORE
    rloc = row % ROWS_PER_CORE
    rbat = rloc % BR                      # row within batch, 0..511
    bat = rloc // BR
    strm = (col >= HALF).astype(np.int64)
    grp = bat * 2 + strm

    order = np.lexsort((rloc, grp, core))
    rbs, cs, vs, gs, cos, ss = (rbat[order], col[order], val[order],
                                grp[order], core[order], strm[order])

    cnt = np.zeros((N_CORES, NG), np.int64)
    np.add.at(cnt, (cos, gs), 1)
    B = (-(-cnt // P)).max(axis=0)        # blocks per group, shared SPMD
    B = np.maximum(B, 1)
    gb = np.zeros(NG + 1, np.int64)
    np.cumsum(B, out=gb[1:])
    TB = int(B.sum())                     # total blocks per core
    slots = TB * P

    # per-edge slot position (rank within its (core, group) run)
    ckey = cos * NG + gs
    starts = np.zeros(E, np.int64)
    newgrp = np.ones(E, bool)
    newgrp[1:] = ckey[1:] != ckey[:-1]
    start_idx = np.where(newgrp)[0]
    starts[start_idx] = start_idx
    starts = np.maximum.accumulate(starts)
    rank = np.arange(E) - starts
    pos = gb[gs] * P + rank
    blk = gb[gs] + rank // P

    idxf = np.zeros((N_CORES, slots), np.int16)
    rowf = np.full((N_CORES, slots), -1024.0, np.float32)
    valf = np.zeros((N_CORES, slots), np.float32)
    idxf[cos, pos] = (cs - ss * HALF).astype(np.int16)
    rowf[cos, pos] = rbs.astype(np.float32)
    valf[cos, pos] = vs

    # per-core block -> window-rel span
    winlo = np.full((N_CORES, TB), WPB, np.int64)
    winhi = np.full((N_CORES, TB), -1, np.int64)
    wrel = rbs // W
    np.minimum.at(winlo, (cos, blk), wrel)
    np.maximum.at(winhi, (cos, blk), wrel)

    # union across cores: M[b, w_rel]
    M = np.zeros((TB, WPB), bool)
    for c in range(N_CORES):
        for w in range(WPB):
            M[:, w] |= (winlo[c] <= w) & (w <= winhi[c])

    # every window needs >=1 task (PSUM init); force via first block of
    # its lo group
    blk_grp = np.repeat(np.arange(NG), B)
    for w_abs in range(WINDOWS):
        b_ = w_abs // WPB
        wr = w_abs % WPB
        sel = (blk_grp % 2 == 0) & (blk_grp // 2 == b_)
        span = M[(blk_grp // 2 == b_), wr]
        if not span.any():
            M[gb[2 * b_], wr] = True

    # ordered task list: group-major, window-major within group, block
    task_block = []
    task_win = []
    grp_task0 = np.zeros(NG + 1, np.int64)
    for g in range(NG):
        b0, b1 = gb[g], gb[g + 1]
        bat_ = g // 2
        for wr in range(WPB):
            bs = np.where(M[b0:b1, wr])[0] + b0
            task_block.extend(bs.tolist())
            task_win.extend([bat_ * WPB + wr] * len(bs))
        grp_task0[g + 1] = len(task_block)
    task_block = np.asarray(task_block, np.int64)
    task_win = np.asarray(task_win, np.int64)
    NT = len(task_block)
    NT_MAX = int((grp_task0[1:] - grp_task0[:-1]).max())

    wft = np.full(WINDOWS, -1, np.int64)
    wlt = np.zeros(WINDOWS, np.int64)
    for t in range(NT):
        w_ = task_win[t]
        if wft[w_] < 0:
            wft[w_] = t
        wlt[w_] = t
    assert (wft >= 0).all()

    # task tables [cores, P, NT] in bf16
    rows3 = rowf.reshape(N_CORES, TB, P)
    vals3 = valf.reshape(N_CORES, TB, P)
    woff = ((task_win % WPB) * W).astype(np.float32)
    rowT = rows3[:, task_block, :].transpose(0, 2, 1) - woff[None, None, :]
    valT = vals3[:, task_block, :].transpose(0, 2, 1)
    rowT = rowT.astype(ml_dtypes.bfloat16)
    valT = valT.astype(ml_dtypes.bfloat16)

    # idx wrapped layout per group chunk, x8 replicated
    S_tot = TB * 8
    idx2d = np.zeros((N_CORES, 16, S_tot), np.int16)
    for g in range(NG):
        b0, b1 = gb[g], gb[g + 1]
        nb = b1 - b0
        seg = idxf[:, b0 * P:b1 * P]
        idx2d[:, :, b0 * 8:b1 * 8] = seg.reshape(
            N_CORES, nb * 8, 16).transpose(0, 2, 1)
    idx2d = np.tile(idx2d, (1, 8, 1))

    return (idx2d, rowT, valT, B, gb, task_block, task_win, grp_task0,
            wft, wlt, TB, NT, NT_MAX, S_tot)


def _build(B, gb, task_block, task_win, grp_task0, wft, wlt, TB, NT,
           NT_MAX, S_tot):
    import concourse.bacc as bacc
    import concourse.mybir as mybir
    from concourse.tile import TileContext

    nc = bacc.Bacc("TRN2", target_bir_lowering=False, debug=False,
                   num_swdge_queues=NQ)
    f32 = mybir.dt.float32
    bf16 = mybir.dt.bfloat16
    xlo = nc.dram_tensor("xlo", [HALF, DP], bf16, kind="ExternalInput")
    xhi = nc.dram_tensor("xhi", [HALF, DP], bf16, kind="ExternalInput")
    idxs = nc.dram_tensor("idxs", [P, S_tot], mybir.dt.int16,
                          kind="ExternalInput")
    rowd = nc.dram_tensor("rowt", [P, NT], bf16, kind="ExternalInput")
    vald = nc.dram_tensor("valt", [P, NT], bf16, kind="ExternalInput")
    out = nc.dram_tensor("out", [ROWS_PER_CORE, D], f32, kind="ExternalOutput")
    xsrc = (xlo, xhi)
    NBMAX = int(B.max())

    with TileContext(nc) as tc:
        with (
            tc.tile_pool(name="meta", bufs=1) as meta,
            tc.tile_pool(name="gat", bufs=12) as gat,
            tc.tile_pool(name="selp", bufs=4) as selp,
            tc.tile_pool(name="psum", bufs=8, space="PSUM") as psp,
            tc.tile_pool(name="ost", bufs=2) as ostp,
        ):
            idx_tile = meta.tile([P, S_tot], mybir.dt.int16)
            n_split = 4
            step = -(-S_tot // n_split)
            for si in range(n_split):
                a, b_ = si * step, min((si + 1) * step, S_tot)
                if a < b_:
                    nc.sync.dma_start(out=idx_tile[:, a:b_], in_=idxs[:, a:b_])
            row_tile = meta.tile([P, NT], bf16)
            nc.sync.dma_start(out=row_tile[:], in_=rowd[:, :])
            val_tile = meta.tile([P, NT], bf16)
            nc.sync.dma_start(out=val_tile[:], in_=vald[:, :])
            iota_i = meta.tile([P, NT_MAX * W], mybir.dt.int32)
            nc.gpsimd.iota(iota_i[:], pattern=[[0, NT_MAX], [1, W]],
                           base=0, channel_multiplier=0)
            iota_f = meta.tile([P, NT_MAX * W], bf16)
            nc.vector.tensor_copy(out=iota_f[:], in_=iota_i[:])

            psum_of = {}
            out_stage = None
            stage_cnt = 0
            qi = 0
            for g in range(NG):
                b0, b1 = int(gb[g]), int(gb[g + 1])
                nb = b1 - b0
                s_ = g % 2
                # gather the group's blocks in chunks of CH_BLOCKS
                blk_tile = {}
                for cb in range(b0, b1, CH_BLOCKS):
                    nbc = min(CH_BLOCKS, b1 - cb)
                    gt = gat.tile([P, CH_BLOCKS * DP], bf16, name="g", tag="g")
                    nc.gpsimd.dma_gather(
                        out_ap=gt[:, :nbc * DP].rearrange(
                            "p (k d) -> p k d", d=DP),
                        in_ap=xsrc[s_][:],
                        idxs_ap=idx_tile[:, cb * 8:(cb + nbc) * 8],
                        num_idxs=nbc * P,
                        num_idxs_reg=nbc * P,
                        elem_size=DP,
                        queue_num=qi % NQ,
                    )
                    qi += 1
                    for j in range(nbc):
                        blk_tile[cb + j] = (gt, j)

                t0, t1 = int(grp_task0[g]), int(grp_task0[g + 1])
                nt = t1 - t0
                selt = selp.tile([P, NT_MAX * W], bf16, name="sel", tag="sel")
                sel3 = selt[:, :nt * W].rearrange("p (t w) -> p t w", w=W)
                nc.vector.tensor_tensor(
                    out=sel3,
                    in0=iota_f[:, :nt * W].rearrange("p (t w) -> p t w", w=W),
                    in1=row_tile[:, t0:t1].to_broadcast([P, nt, W]),
                    op=mybir.AluOpType.is_equal,
                )
                nc.vector.tensor_tensor(
                    out=sel3,
                    in0=sel3,
                    in1=val_tile[:, t0:t1].to_broadcast([P, nt, W]),
                    op=mybir.AluOpType.mult,
                )

                for t in range(t0, t1):
                    b = int(task_block[t])
                    w_ = int(task_win[t])
                    bt, j = blk_tile[b]
                    if w_ not in psum_of:
                        psum_of[w_] = psp.tile([W, D], f32, name="psw", tag="psw")
                    nc.tensor.matmul(
                        out=psum_of[w_][:, :],
                        lhsT=selt[:, (t - t0) * W:(t - t0 + 1) * W],
                        rhs=bt[:, j * DP:j * DP + D],
                        start=(t == wft[w_]),
                        stop=(t == wlt[w_]),
                    )
                    if t == wlt[w_]:
                        wi = w_ % WPB
                        if stage_cnt == 0:
                            out_stage = ostp.tile([W, WPB * D], f32, name="ostage")
                        nc.scalar.copy(out=out_stage[:, wi * D:(wi + 1) * D],
                                       in_=psum_of.pop(w_)[:, :])
                        stage_cnt += 1
                        if stage_cnt == WPB:
                            stage_cnt = 0
                            bat_ = w_ // WPB
                            dview = out[bat_ * BR:(bat_ + 1) * BR, :].rearrange(
                                "(g p) f -> p g f", p=W)
                            sview = out_stage[:].rearrange(
                                "p (g f) -> p g f", f=D)
                            nc.sync.dma_start(out=dview, in_=sview)
    nc.compile()
    return nc


def kernel(x, row, col, val, idx):
    global LAST_EXEC_NS
    from concourse.bass_utils import run_bass_kernel_spmd

    x = np.ascontiguousarray(np.asarray(x), dtype=np.float32)
    row = np.asarray(row).astype(np.int64)
    col = np.asarray(col).astype(np.int64)
    val = np.ascontiguousarray(np.asarray(val), dtype=np.float32)

    (idx2d, rowT, valT, B, gb, task_block, task_win, grp_task0,
     wft, wlt, TB, NT, NT_MAX, S_tot) = _pack(row, col, val)
    nc = _build(B, gb, task_block, task_win, grp_task0, wft, wlt, TB,
                NT, NT_MAX, S_tot)

    xpad = np.zeros((N_NODES, DP), ml_dtypes.bfloat16)
    xpad[:, :D] = x.astype(ml_dtypes.bfloat16)
    xlo = np.ascontiguousarray(xpad[:HALF])
    xhi = np.ascontiguousarray(xpad[HALF:])
    in_maps = [
        {"xlo": xlo, "xhi": xhi, "idxs": idx2d[c], "rowt": rowT[c],
         "valt": valT[c]}
        for c in range(N_CORES)
    ]
    trace = os.environ.get("BASS_KERNEL_TRACE", "0") == "1"
    res = run_bass_kernel_spmd(nc, in_maps, list(range(N_CORES)), trace=trace)
    LAST_EXEC_NS = res.exec_time_ns
    outs = [np.asarray(res.results[c]["out"]) for c in range(N_CORES)]
    return np.concatenate(outs, axis=0)


# revision 8
# speedup vs baseline: 1.4335x; 1.1792x over previous
"""SpMM (COO segment-sum) kernel for trn2, 8 NeuronCores.

out[i] = sum_{e: row[e]==i} val[e] * x[col[e]]   (N=65536, E~1M, D=64)

Strategy (dest-row 1D sharding per spec hint), v3:
- Host: shard rows 8192/core; within a core, bucket edges into 16
  batches of 512 rows x 2 column streams (col<32768 / col>=32768 so
  node indices fit dma_gather's int16), sort each (batch, stream)
  group by row and pack densely into 128-edge blocks (pad only the
  group tail; block counts maxed across cores -> ~4% padding vs the
  window-aligned baseline's 150%).
- A block's rows may span several 64-row PSUM windows. Host emits
  (block, window) "tasks" as the union of spans across cores; on a
  core where the block misses the window, row-rel values fall outside
  [0,64) and the one-hot select is all zero, so the matmul is a no-op.
- Device, per chunk of <=8 blocks: one 1024-idx dma_gather (f32 x,
  256B rows; single-packet SWDGE is the fast path), ACT casts the
  chunk to bf16. Per group: sel built in a "vertical" [P, W, nt]
  layout so both DVE tensor_tensor operands have stride-1 bf16 last
  dims (2x_1p DVE mode):
    sel[p, w*NTM+t] = (iota_w == rowT[p, t]) * valT[p, t]
  Per task: bf16 matmul with a strided lhsT column view,
    psum[win][r, f] += sum_p sel[p, r*NTM+ti] * gb[p, f].
  PSUM windows drain via ACT copy to SBUF, one output DMA per batch.
"""

import os
import numpy as np
import ml_dtypes

N_NODES = 65536
D = 64
P = 128
N_CORES = 8
ROWS_PER_CORE = N_NODES // N_CORES   # 8192
W = 64                               # rows per PSUM window
WINDOWS = ROWS_PER_CORE // W         # 128
WPB = 8                              # windows per batch
BR = W * WPB                         # rows per batch (512)
NBATCH = ROWS_PER_CORE // BR         # 16
NG = NBATCH * 2                      # (batch, stream) groups
HALF = N_NODES // 2
NQ = 4                               # SWDGE queues
CH = 8                               # blocks per dma_gather chunk (1024 idx)

LAST_EXEC_NS = None


def _pack(row, col, val):
    """Host-side packing. Returns per-core device arrays + shared program."""
    E = row.shape[0]
    core = row // ROWS_PER_CORE
    rloc = row % ROWS_PER_CORE
    rbat = rloc % BR                      # row within batch, 0..511
    bat = rloc // BR
    strm = (col >= HALF).astype(np.int64)
    grp = bat * 2 + strm

    order = np.lexsort((rloc, grp, core))
    rbs, cs, vs, gs, cos, ss = (rbat[order], col[order], val[order],
                                grp[order], core[order], strm[order])

    cnt = np.zeros((N_CORES, NG), np.int64)
    np.add.at(cnt, (cos, gs), 1)
    B = (-(-cnt // P)).max(axis=0)        # blocks per group, shared SPMD
    B = np.maximum(B, 1)
    gb = np.zeros(NG + 1, np.int64)
    np.cumsum(B, out=gb[1:])
    TB = int(B.sum())
    slots = TB * P

    ckey = cos * NG + gs
    starts = np.zeros(E, np.int64)
    newgrp = np.ones(E, bool)
    newgrp[1:] = ckey[1:] != ckey[:-1]
    start_idx = np.where(newgrp)[0]
    starts[start_idx] = start_idx
    starts = np.maximum.accumulate(starts)
    rank = np.arange(E) - starts
    pos = gb[gs] * P + rank
    blk = gb[gs] + rank // P

    idxf = np.zeros((N_CORES, slots), np.int16)
    rowf = np.full((N_CORES, slots), -1024.0, np.float32)
    valf = np.zeros((N_CORES, slots), np.float32)
    idxf[cos, pos] = (cs - ss * HALF).astype(np.int16)
    rowf[cos, pos] = rbs.astype(np.float32)
    valf[cos, pos] = vs

    # per-core block -> window-rel span, then union across cores
    winlo = np.full((N_CORES, TB), WPB, np.int64)
    winhi = np.full((N_CORES, TB), -1, np.int64)
    wrel = rbs // W
    np.minimum.at(winlo, (cos, blk), wrel)
    np.maximum.at(winhi, (cos, blk), wrel)
    M = np.zeros((TB, WPB), bool)
    for c in range(N_CORES):
        for w in range(WPB):
            M[:, w] |= (winlo[c] <= w) & (w <= winhi[c])

    # every window needs >=1 task (PSUM init)
    blk_grp = np.repeat(np.arange(NG), B)
    for w_abs in range(WINDOWS):
        b_ = w_abs // WPB
        wr = w_abs % WPB
        if not M[(blk_grp // 2 == b_), wr].any():
            M[gb[2 * b_], wr] = True

    # ordered task list: group-major, window-major within group
    task_block = []
    task_win = []
    grp_task0 = np.zeros(NG + 1, np.int64)
    for g in range(NG):
        b0, b1 = gb[g], gb[g + 1]
        bat_ = g // 2
        for wr in range(WPB):
            bs = np.where(M[b0:b1, wr])[0] + b0
            task_block.extend(bs.tolist())
            task_win.extend([bat_ * WPB + wr] * len(bs))
        grp_task0[g + 1] = len(task_block)
    task_block = np.asarray(task_block, np.int64)
    task_win = np.asarray(task_win, np.int64)
    NT = len(task_block)
    NT_MAX = int((grp_task0[1:] - grp_task0[:-1]).max())

    wft = np.full(WINDOWS, -1, np.int64)
    wlt = np.zeros(WINDOWS, np.int64)
    for t in range(NT):
        w_ = task_win[t]
        if wft[w_] < 0:
            wft[w_] = t
        wlt[w_] = t
    assert (wft >= 0).all()

    rows3 = rowf.reshape(N_CORES, TB, P)
    vals3 = valf.reshape(N_CORES, TB, P)
    woff = ((task_win % WPB) * W).astype(np.float32)
    rowT = rows3[:, task_block, :].transpose(0, 2, 1) - woff[None, None, :]
    valT = vals3[:, task_block, :].transpose(0, 2, 1)
    rowT = np.ascontiguousarray(rowT).astype(ml_dtypes.bfloat16)
    valT = np.ascontiguousarray(valT).astype(ml_dtypes.bfloat16)

    # idx wrapped layout per gather chunk (CH blocks), x8 replicated
    S_tot = TB * 8
    idx2d = np.zeros((N_CORES, 16, S_tot), np.int16)
    for g in range(NG):
        b0, b1 = int(gb[g]), int(gb[g + 1])
        for cb in range(b0, b1, CH):
            nb = min(CH, b1 - cb)
            seg = idxf[:, cb * P:(cb + nb) * P]
            idx2d[:, :, cb * 8:(cb + nb) * 8] = seg.reshape(
                N_CORES, nb * 8, 16).transpose(0, 2, 1)
    idx2d = np.tile(idx2d, (1, 8, 1))

    return (idx2d, rowT, valT, B, gb, task_block, task_win, grp_task0,
            wft, wlt, TB, NT, NT_MAX, S_tot)


def _build(B, gb, task_block, task_win, grp_task0, wft, wlt, TB, NT,
           NT_MAX, S_tot):
    import concourse.bacc as bacc
    import concourse.mybir as mybir
    from concourse.tile import TileContext

    nc = bacc.Bacc("TRN2", target_bir_lowering=False, debug=False,
                   num_swdge_queues=NQ)
    f32 = mybir.dt.float32
    bf16 = mybir.dt.bfloat16
    xlo = nc.dram_tensor("xlo", [HALF, D], f32, kind="ExternalInput")
    xhi = nc.dram_tensor("xhi", [HALF, D], f32, kind="ExternalInput")
    idxs = nc.dram_tensor("idxs", [P, S_tot], mybir.dt.int16,
                          kind="ExternalInput")
    rowd = nc.dram_tensor("rowt", [P, NT], bf16, kind="ExternalInput")
    vald = nc.dram_tensor("valt", [P, NT], bf16, kind="ExternalInput")
    out = nc.dram_tensor("out", [ROWS_PER_CORE, D], f32, kind="ExternalOutput")
    xsrc = (xlo, xhi)
    NBMAX = int(B.max())

    with TileContext(nc) as tc:
        with (
            tc.tile_pool(name="meta", bufs=1) as meta,
            tc.tile_pool(name="gat", bufs=12) as gat,
            tc.tile_pool(name="gbp", bufs=3) as gbp,
            tc.tile_pool(name="selp", bufs=3) as selp,
            tc.tile_pool(name="psum", bufs=8, space="PSUM") as psp,
            tc.tile_pool(name="ost", bufs=2) as ostp,
        ):
            idx_tile = meta.tile([P, S_tot], mybir.dt.int16)
            n_split = 4
            step = -(-S_tot // n_split)
            for si in range(n_split):
                a, b_ = si * step, min((si + 1) * step, S_tot)
                if a < b_:
                    nc.sync.dma_start(out=idx_tile[:, a:b_], in_=idxs[:, a:b_])
            row_tile = meta.tile([P, NT], bf16)
            nc.sync.dma_start(out=row_tile[:], in_=rowd[:, :])
            val_tile = meta.tile([P, NT], bf16)
            nc.sync.dma_start(out=val_tile[:], in_=vald[:, :])
            # vertical iota: iota_v[p, w*NT_MAX + t] = w
            iota_i = meta.tile([P, W * NT_MAX], mybir.dt.int32)
            nc.gpsimd.iota(iota_i[:], pattern=[[1, W], [0, NT_MAX]],
                           base=0, channel_multiplier=0)
            iota_f = meta.tile([P, W * NT_MAX], bf16)
            nc.vector.tensor_copy(out=iota_f[:], in_=iota_i[:])

            psum_of = {}
            out_stage = None
            stage_cnt = 0
            qi = 0
            for g in range(NG):
                b0, b1 = int(gb[g]), int(gb[g + 1])
                nb = b1 - b0
                s_ = g % 2
                # gather chunks (f32, 256B rows), cast each to bf16 on ACT
                gb_tile = gbp.tile([P, NBMAX * D], bf16, name="gb", tag="gb")
                for cb in range(b0, b1, CH):
                    nbc = min(CH, b1 - cb)
                    gt = gat.tile([P, CH * D], f32, name="g", tag="g")
                    nc.gpsimd.dma_gather(
                        out_ap=gt[:, :nbc * D].rearrange(
                            "p (k d) -> p k d", d=D),
                        in_ap=xsrc[s_][:],
                        idxs_ap=idx_tile[:, cb * 8:(cb + nbc) * 8],
                        num_idxs=nbc * P,
                        num_idxs_reg=nbc * P,
                        elem_size=D,
                        queue_num=qi % NQ,
                    )
                    qi += 1
                    nc.scalar.copy(
                        out=gb_tile[:, (cb - b0) * D:(cb - b0 + nbc) * D],
                        in_=gt[:, :nbc * D])

                t0, t1 = int(grp_task0[g]), int(grp_task0[g + 1])
                nt = t1 - t0
                selt = selp.tile([P, W * NT_MAX], bf16, name="sel", tag="sel")
                selv = selt[:].rearrange("p (w t) -> p w t", t=NT_MAX)
                sel3 = selv[:, :, :nt]
                rbc = row_tile[:, t0:t1].rearrange(
                    "p (o t) -> p o t", o=1).broadcast_to([P, W, nt])
                vbc = val_tile[:, t0:t1].rearrange(
                    "p (o t) -> p o t", o=1).broadcast_to([P, W, nt])
                nc.vector.tensor_tensor(
                    out=sel3,
                    in0=iota_f[:].rearrange("p (w t) -> p w t",
                                            t=NT_MAX)[:, :, :nt],
                    in1=rbc,
                    op=mybir.AluOpType.is_equal,
                )
                nc.vector.tensor_tensor(
                    out=sel3, in0=sel3, in1=vbc, op=mybir.AluOpType.mult,
                )

                for t in range(t0, t1):
                    b = int(task_block[t])
                    w_ = int(task_win[t])
                    j = b - b0
                    ti = t - t0
                    if w_ not in psum_of:
                        psum_of[w_] = psp.tile([W, D], f32, name="psw",
                                               tag="psw")
                    nc.tensor.matmul(
                        out=psum_of[w_][:, :],
                        lhsT=selv[:, :, ti],
                        rhs=gb_tile[:, j * D:(j + 1) * D],
                        start=(t == wft[w_]),
                        stop=(t == wlt[w_]),
                    )
                    if t == wlt[w_]:
                        wi = w_ % WPB
                        if stage_cnt == 0:
                            out_stage = ostp.tile([W, WPB * D], f32,
                                                  name="ostage")
                        nc.scalar.copy(out=out_stage[:, wi * D:(wi + 1) * D],
                                       in_=psum_of.pop(w_)[:, :])
                        stage_cnt += 1
                        if stage_cnt == WPB:
                            stage_cnt = 0
                            bat_ = w_ // WPB
                            dview = out[bat_ * BR:(bat_ + 1) * BR, :].rearrange(
                                "(g p) f -> p g f", p=W)
                            sview = out_stage[:].rearrange(
                                "p (g f) -> p g f", f=D)
                            nc.sync.dma_start(out=dview, in_=sview)
    nc.compile()
    return nc


def kernel(x, row, col, val, idx):
    global LAST_EXEC_NS
    from concourse.bass_utils import run_bass_kernel_spmd

    x = np.ascontiguousarray(np.asarray(x), dtype=np.float32)
    row = np.asarray(row).astype(np.int64)
    col = np.asarray(col).astype(np.int64)
    val = np.ascontiguousarray(np.asarray(val), dtype=np.float32)

    (idx2d, rowT, valT, B, gb, task_block, task_win, grp_task0,
     wft, wlt, TB, NT, NT_MAX, S_tot) = _pack(row, col, val)
    nc = _build(B, gb, task_block, task_win, grp_task0, wft, wlt, TB,
                NT, NT_MAX, S_tot)

    xlo = np.ascontiguousarray(x[:HALF])
    xhi = np.ascontiguousarray(x[HALF:])
    in_maps = [
        {"xlo": xlo, "xhi": xhi, "idxs": idx2d[c], "rowt": rowT[c],
         "valt": valT[c]}
        for c in range(N_CORES)
    ]
    trace = os.environ.get("BASS_KERNEL_TRACE", "0") == "1"
    res = run_bass_kernel_spmd(nc, in_maps, list(range(N_CORES)), trace=trace)
    LAST_EXEC_NS = res.exec_time_ns
    outs = [np.asarray(res.results[c]["out"]) for c in range(N_CORES)]
    return np.concatenate(outs, axis=0)
